# revision 8
# baseline (speedup 1.0000x reference)
"""Trainium2 Bass kernel for nn_BidirectionalNeuralSymbolic (vq_codebook).

Model (per batch row x of dim 1024):
  encoded = relu(x @ W1 + b1) @ W2 + b2                      # [E=128]
  sims    = cos(encoded, protos[k]) for k in 4096            # [K]
  concept_probs = softmax(10 * sims)                         # output 1 [B,K]
  best = argmax(concept_probs); bp = protos[best]            # gather
  ai = [encoded, bp]                                         # [256]
  abstraction = relu(ai @ W3 + b3) @ W4 + b4                 # output 2 [B,E]
  hierarchy   = sigmoid(ai @ Wh.T + bh)                      # output 3 [B,L]

Sharding: pure data-parallel over batch across 8 NeuronCores; all params
replicated. Each core handles 4096 rows as 8 blocks of 512 (4 sub-tiles
of 128) so the heavy matmuls run with a 512-wide moving operand.

Host-side prep inside kernel(): per-core batch slices are pre-transposed
([D_IN, 4096], the layout the feature-major matmuls consume) and the
codebook is pre-normalized/transposed once ([E, K]); raw protos stay in
DRAM for the argmax gather. All on-device math is fp32 — the top-2 sims
gap is as small as 9e-7 on these inputs, so any reduced-precision matmul
would flip argmax rows and corrupt `abstraction`.

Engine plan per 512-row block:
  PE : H1 (32 mm N=512), encoded^T (4 mm N=512), row-norm ones-matmuls,
       sims (32 mm N=512), best-proto transposes, abstraction/hierarchy.
  ACT: PSUM evacuations fused with bias/ReLU; exp of sims over [128,1024]
       PSUM spans with the per-row 10/||enc|| as the per-partition scale
       and accum_out producing softmax denominators for free. Only
       EXP-set table functions are used -> single ACT table load.
  DVE: max8 + max_index argmax (first-index ties = jnp.argmax), smalls.
  GPS: rsqrt via pow(s,-0.5) (Q7 vpowf), softmax divide (tensor_scalar
       by 1/sum, exact), indirect-DMA gather of argmax protos rows.
PSUM banks: h1 2 + e0/ns 1 + sims 4 + late 1 = 8.
"""

import numpy as np
from contextlib import ExitStack

import concourse.bass as bass
import concourse.mybir as mybir
import concourse.tile as tile
from concourse import bacc
from concourse.bass_utils import run_bass_kernel_spmd
from concourse.masks import make_identity

F32 = mybir.dt.float32
U32 = mybir.dt.uint32
AF = mybir.ActivationFunctionType
OP = mybir.AluOpType

B, D_IN, K, E, L, H = 32768, 1024, 4096, 128, 5, 512
N_CORES = 8
BC = B // N_CORES          # 4096 rows per core
P = 128
SUB = 4                    # 128-row sub-tiles per block
BLK = SUB * P              # 512 rows per block
NB = BC // BLK             # 8 blocks per core
KC = D_IN // P             # 8 contraction chunks for layer 1
HC = H // P                # 4 hidden chunks
SC = 4                     # sims spans of 1024 per sub-tile
SW = K // SC               # 1024 (two N=512 matmuls + one exp each)

_cached = None
last_results = None


def _build():
    nc = bacc.Bacc("TRN2", target_bir_lowering=False, debug=False)

    xT_d = nc.dram_tensor("inputsT", [D_IN, BC], F32, kind="ExternalInput")
    w1_d = nc.dram_tensor("W1", [D_IN, H], F32, kind="ExternalInput")
    b1_d = nc.dram_tensor("b1", [H], F32, kind="ExternalInput")
    w2_d = nc.dram_tensor("W2", [H, E], F32, kind="ExternalInput")
    b2_d = nc.dram_tensor("b2", [E], F32, kind="ExternalInput")
    pr_d = nc.dram_tensor("protos", [K, E], F32, kind="ExternalInput")
    pnT_d = nc.dram_tensor("pro_nT", [E, K], F32, kind="ExternalInput")
    w3_d = nc.dram_tensor("W3", [2 * E, E], F32, kind="ExternalInput")
    b3_d = nc.dram_tensor("b3", [E], F32, kind="ExternalInput")
    w4_d = nc.dram_tensor("W4", [E, E], F32, kind="ExternalInput")
    b4_d = nc.dram_tensor("b4", [E], F32, kind="ExternalInput")
    wh_d = nc.dram_tensor("Wh", [L, 2 * E], F32, kind="ExternalInput")
    bh_d = nc.dram_tensor("bh", [L], F32, kind="ExternalInput")

    probs_d = nc.dram_tensor("probs", [BC, K], F32, kind="ExternalOutput")
    abst_d = nc.dram_tensor("abst", [BC, E], F32, kind="ExternalOutput")
    hier_d = nc.dram_tensor("hier", [BC, L], F32, kind="ExternalOutput")

    with tile.TileContext(nc) as tc, ExitStack() as ctx:
        const = ctx.enter_context(tc.tile_pool(name="const", bufs=1))
        work = ctx.enter_context(tc.tile_pool(name="work", bufs=3))
        bigp = ctx.enter_context(tc.tile_pool(name="bigp", bufs=2))
        expp = ctx.enter_context(tc.tile_pool(name="expp", bufs=3))
        ps_h1 = ctx.enter_context(tc.tile_pool(name="ps_h1", bufs=2, space="PSUM"))
        ps_en = ctx.enter_context(tc.tile_pool(name="ps_en", bufs=1, space="PSUM"))
        ps_si = ctx.enter_context(tc.tile_pool(name="ps_si", bufs=2, space="PSUM"))
        ps_lt = ctx.enter_context(tc.tile_pool(name="ps_lt", bufs=1, space="PSUM"))

        # ---------------- prologue: constants (DMAs only) ----------------
        ident = const.tile([P, P], F32)
        make_identity(nc, ident[:])
        ones = const.tile([P, 1], F32)
        nc.vector.memset(ones[:], 1.0)
        nhalf = const.tile([P, SUB], F32)
        nc.vector.memset(nhalf[:], -0.5)

        w1sb = const.tile([P, KC * H], F32)       # block kc at [:, kc*512:...]
        for kc in range(KC):
            nc.sync.dma_start(
                w1sb[:, kc * H:(kc + 1) * H], w1_d[kc * P:(kc + 1) * P, :]
            )
        w2sb = const.tile([P, HC * E], F32)
        for hc in range(HC):
            nc.sync.dma_start(
                w2sb[:, hc * E:(hc + 1) * E], w2_d[hc * P:(hc + 1) * P, :]
            )
        b1c = const.tile([P, HC], F32)
        nc.sync.dma_start(b1c[:], b1_d.ap().rearrange("(c p) -> p c", p=P))
        b2c = const.tile([P, 1], F32)
        nc.sync.dma_start(b2c[:], b2_d.ap().rearrange("(c p) -> p c", p=P))

        # needed only from the sims stage on — emitted after the first
        # encoder DMAs so they don't delay the first H1 matmuls
        pro_nT = const.tile([P, K], F32)
        nc.sync.dma_start(pro_nT[:], pnT_d[:])
        w3sb = const.tile([P, 2 * E], F32)
        for c in range(2):
            nc.sync.dma_start(
                w3sb[:, c * E:(c + 1) * E], w3_d[c * P:(c + 1) * P, :]
            )
        w4sb = const.tile([P, E], F32)
        nc.sync.dma_start(w4sb[:], w4_d[:])
        whT = const.tile([P, 2 * L], F32)         # chunk c at [:, c*L:(c+1)*L]
        for c in range(2):
            nc.sync.dma_start(
                whT[:, c * L:(c + 1) * L],
                wh_d[:, c * P:(c + 1) * P].rearrange("l p -> p l"),
            )
        b3c = const.tile([P, 1], F32)
        nc.sync.dma_start(b3c[:], b3_d.ap().rearrange("(c p) -> p c", p=P))
        b4row = const.tile([1, E], F32)
        nc.sync.dma_start(b4row[:], b4_d.ap().rearrange("(o f) -> o f", o=1))
        b4b = const.tile([P, E], F32)
        nc.gpsimd.partition_broadcast(b4b[:], b4row[:])
        bhrow = const.tile([1, L], F32)
        nc.sync.dma_start(bhrow[:], bh_d.ap().rearrange("(o f) -> o f", o=1))
        bhb = const.tile([P, L], F32)
        nc.gpsimd.partition_broadcast(bhb[:], bhrow[:])

        def emit_late(e0_, s, sr0, bp):
            """abstraction + hierarchy for one sub-tile (runs one sub late
            so the PE never stalls on the gather's DMA semaphore)."""
            bpt_ps = ps_lt.tile([P, P], F32, tag="lt")
            nc.tensor.transpose(bpt_ps[:], bp[:], ident[:])
            bpts = work.tile([P, E], F32, tag="bpts")
            nc.scalar.copy(bpts[:], bpt_ps[:])

            a1_ps = ps_lt.tile([P, E], F32, tag="lt")
            nc.tensor.matmul(
                a1_ps[:], w3sb[:, :E], e0_[:, s * P:(s + 1) * P],
                start=True, stop=False,
            )
            nc.tensor.matmul(
                a1_ps[:], w3sb[:, E:], bpts[:], start=False, stop=True
            )
            a1 = work.tile([P, E], F32, tag="a1")
            nc.scalar.activation(a1[:], a1_ps[:], AF.Relu, bias=b3c[:, :1])
            ab_ps = ps_lt.tile([P, E], F32, tag="lt")
            nc.tensor.matmul(ab_ps[:], a1[:], w4sb[:], start=True, stop=True)
            ab = work.tile([P, E], F32, tag="ab")
            nc.vector.tensor_tensor(
                out=ab[:], in0=ab_ps[:], in1=b4b[:], op=OP.add
            )
            nc.sync.dma_start(abst_d[sr0:sr0 + P, :], ab[:])

            hr_ps = ps_lt.tile([P, L], F32, tag="lt")
            nc.tensor.matmul(
                hr_ps[:], e0_[:, s * P:(s + 1) * P], whT[:, :L],
                start=True, stop=False,
            )
            nc.tensor.matmul(
                hr_ps[:], bpts[:], whT[:, L:], start=False, stop=True
            )
            hz = work.tile([P, L], F32, tag="hz")
            nc.vector.tensor_tensor(
                out=hz[:], in0=hr_ps[:], in1=bhb[:], op=OP.add
            )
            he = work.tile([P, L], F32, tag="he")
            nc.scalar.activation(he[:], hz[:], AF.Exp, scale=-1.0)
            hd = work.tile([P, L], F32, tag="hd")
            nc.vector.tensor_scalar(
                out=hd[:], in0=he[:], scalar1=1.0, scalar2=None, op0=OP.add,
            )
            hs = work.tile([P, L], F32, tag="hs")
            nc.vector.reciprocal(hs[:], hd[:])
            nc.sync.dma_start(hier_d[sr0:sr0 + P, :], hs[:])

        pending = None

        # ---------------- main loop over 8 blocks of 512 rows ----------------
        for b in range(NB):
            r0 = b * BLK
            # x^T feature-major straight from DRAM (host pre-transposed)
            xt = bigp.tile([P, KC * BLK], F32, tag="xt")
            for kc in range(KC):
                nc.sync.dma_start(
                    xt[:, kc * BLK:(kc + 1) * BLK],
                    xT_d[kc * P:(kc + 1) * P, r0:r0 + BLK],
                )

            # H1^T = relu(W1^T x^T + b1): 4 chunks of [128h, 512b]
            h1 = bigp.tile([P, HC * BLK], F32, tag="h1")
            for hc in range(HC):
                h1_ps = ps_h1.tile([P, BLK], F32, tag="h1p")
                for kc in range(KC):
                    nc.tensor.matmul(
                        h1_ps[:],
                        w1sb[:, kc * H + hc * P: kc * H + (hc + 1) * P],
                        xt[:, kc * BLK:(kc + 1) * BLK],
                        start=(kc == 0), stop=(kc == KC - 1),
                    )
                nc.scalar.activation(
                    h1[:, hc * BLK:(hc + 1) * BLK], h1_ps[:],
                    AF.Relu, bias=b1c[:, hc:hc + 1],
                )

            # encoded^T [E, 512b]
            e0_ps = ps_en.tile([P, BLK], F32, tag="en")
            for hc in range(HC):
                nc.tensor.matmul(
                    e0_ps[:], w2sb[:, hc * E:(hc + 1) * E],
                    h1[:, hc * BLK:(hc + 1) * BLK],
                    start=(hc == 0), stop=(hc == HC - 1),
                )
            e0 = work.tile([P, BLK], F32, tag="e0")
            nc.scalar.activation(e0[:], e0_ps[:], AF.Identity, bias=b2c[:, :1])

            # per-row norms: [b,1] per sub via ones-matmul, packed [128, SUB]
            e0sq = work.tile([P, BLK], F32, tag="e0sq")
            nc.scalar.activation(e0sq[:], e0[:], AF.Square)
            ns_ps = ps_en.tile([P, SUB], F32, tag="en")
            for s in range(SUB):
                nc.tensor.matmul(
                    ns_ps[:, s:s + 1], e0sq[:, s * P:(s + 1) * P], ones[:],
                    start=True, stop=True,
                )
            ns = work.tile([P, SUB], F32, tag="ns")
            nc.vector.tensor_copy(ns[:], ns_ps[:])
            # scale = 10 * min(pow(s, -0.5), 1e8)
            inv = work.tile([P, SUB], F32, tag="inv")
            nc.gpsimd.tensor_tensor(out=inv[:], in0=ns[:], in1=nhalf[:], op=OP.pow)
            scl = work.tile([P, SUB], F32, tag="scl")
            nc.vector.tensor_scalar(
                out=scl[:], in0=inv[:], scalar1=1e8, scalar2=10.0,
                op0=OP.min, op1=OP.mult,
            )

            for s in range(SUB):
                sr0 = r0 + s * P
                # sims -> exp(scale * sims); accum_out -> span sums
                ex = expp.tile([P, K], F32, tag="ex")
                s4 = work.tile([P, SC], F32, tag="s4")
                for c in range(SC):
                    si_ps = ps_si.tile([P, SW], F32, tag="si")
                    for half in range(2):
                        nc.tensor.matmul(
                            si_ps[:, half * 512:(half + 1) * 512],
                            e0[:, s * P:(s + 1) * P],
                            pro_nT[:, c * SW + half * 512: c * SW + (half + 1) * 512],
                            start=True, stop=True,
                        )
                    nc.scalar.activation(
                        ex[:, c * SW:(c + 1) * SW], si_ps[:], AF.Exp,
                        scale=scl[:, s:s + 1], accum_out=s4[:, c:c + 1],
                    )
                ssum = work.tile([P, 1], F32, tag="ssum")
                nc.vector.reduce_sum(
                    out=ssum[:], in_=s4[:], axis=mybir.AxisListType.X
                )
                rs = work.tile([P, 1], F32, tag="rs")
                nc.vector.reciprocal(rs[:], ssum[:])

                # argmax over exp values (same ordering as probs)
                mx8 = work.tile([P, 8], F32, tag="mx8")
                nc.vector.max(out=mx8[:], in_=ex[:])
                mi8 = work.tile([P, 8], U32, tag="mi8")
                nc.vector.max_index(out=mi8[:], in_max=mx8[:], in_values=ex[:])
                bp = work.tile([P, E], F32, tag="bp")
                nc.gpsimd.indirect_dma_start(
                    out=bp[:], out_offset=None, in_=pr_d[:],
                    in_offset=bass.IndirectOffsetOnAxis(ap=mi8[:, :1], axis=0),
                )

                # probs = ex / sum  (in place), then DMA out
                nc.vector.tensor_scalar(
                    out=ex[:], in0=ex[:], scalar1=rs[:, :1], scalar2=None,
                    op0=OP.mult,
                )
                nc.sync.dma_start(probs_d[sr0:sr0 + P, :], ex[:])

                # late chain for the PREVIOUS sub-tile (gather now done)
                if pending is not None:
                    emit_late(*pending)
                pending = (e0, s, sr0, bp)

        if pending is not None:
            emit_late(*pending)

    nc.compile()
    return nc


def _prep_in_maps(inputs):
    full = {k: np.ascontiguousarray(np.asarray(v, dtype=np.float32))
            for k, v in inputs.items()}
    x = full.pop("inputs")
    protos = full["protos"]
    norms = np.maximum(
        np.linalg.norm(protos.astype(np.float64), axis=1, keepdims=True), 1e-8
    )
    full["pro_nT"] = np.ascontiguousarray(
        (protos.astype(np.float64) / norms).T.astype(np.float32)
    )
    xT = x.T  # [D_IN, B]
    in_maps = []
    for c in range(N_CORES):
        m = dict(full)
        m["inputsT"] = np.ascontiguousarray(xT[:, c * BC:(c + 1) * BC])
        in_maps.append(m)
    return in_maps


def kernel(**inputs):
    global _cached, last_results
    if _cached is None:
        _cached = _build()
    res = run_bass_kernel_spmd(_cached, _prep_in_maps(inputs),
                               list(range(N_CORES)))
    last_results = res
    probs = np.concatenate([r["probs"] for r in res.results], axis=0)
    abst = np.concatenate([r["abst"] for r in res.results], axis=0)
    hier = np.concatenate([r["hier"] for r in res.results], axis=0)
    return probs, abst, hier


def run_traced(inputs):
    """Profiled run (test-harness helper; requires the axon NTFF hook)."""
    global _cached
    if _cached is None:
        _cached = _build()
    return run_bass_kernel_spmd(_cached, _prep_in_maps(inputs),
                                list(range(N_CORES)), trace=True)


# revision 9
# speedup vs baseline: 1.2826x; 1.2826x over previous
"""Trainium2 Bass kernel for nn_BidirectionalNeuralSymbolic (vq_codebook).

Model (per batch row x of dim 1024):
  encoded = relu(x @ W1 + b1) @ W2 + b2                      # [E=128]
  sims    = cos(encoded, protos[k]) for k in 4096            # [K]
  concept_probs = softmax(10 * sims)                         # output 1 [B,K]
  best = argmax(concept_probs); bp = protos[best]            # gather
  ai = [encoded, bp]                                         # [256]
  abstraction = relu(ai @ W3 + b3) @ W4 + b4                 # output 2 [B,E]
  hierarchy   = sigmoid(ai @ Wh.T + bh)                      # output 3 [B,L]

Sharding: pure data-parallel over batch across 8 NeuronCores; all params
replicated. Each core handles 4096 rows as 8 blocks of 512 (4 sub-tiles
of 128) so the heavy matmuls run with a 512-wide moving operand.

Host-side prep inside kernel(): per-core batch slices are pre-transposed
([D_IN, 4096], the layout the feature-major matmuls consume) and the
codebook is pre-normalized/transposed once ([E, K]); raw protos stay in
DRAM for the argmax gather. All on-device math is fp32 — the top-2 sims
gap is as small as 9e-7 on these inputs, so any reduced-precision matmul
would flip argmax rows and corrupt `abstraction`.

Engine plan per 512-row block:
  PE : H1 (32 mm N=512), encoded^T (4 mm N=512), row-norm ones-matmuls,
       sims (32 mm N=512), best-proto transposes, abstraction/hierarchy.
  ACT: PSUM evacuations fused with bias/ReLU; exp of sims over [128,1024]
       PSUM spans with the per-row 10/||enc|| as the per-partition scale
       and accum_out producing softmax denominators for free. Only
       EXP-set table functions are used -> single ACT table load.
  DVE: max8 + max_index argmax (first-index ties = jnp.argmax), smalls.
  GPS: rsqrt via pow(s,-0.5) (Q7 vpowf), softmax divide (tensor_scalar
       by 1/sum, exact), indirect-DMA gather of argmax protos rows.
PSUM banks: h1 2 + e0/ns 1 + sims 4 + late 1 = 8.
"""

import numpy as np
from contextlib import ExitStack

import concourse.bass as bass
import concourse.mybir as mybir
import concourse.tile as tile
from concourse import bacc
from concourse.bass_utils import run_bass_kernel_spmd
from concourse.masks import make_identity

F32 = mybir.dt.float32
U32 = mybir.dt.uint32
AF = mybir.ActivationFunctionType
OP = mybir.AluOpType

B, D_IN, K, E, L, H = 32768, 1024, 4096, 128, 5, 512
N_CORES = 8
BC = B // N_CORES          # 4096 rows per core
P = 128
SUB = 4                    # 128-row sub-tiles per block
BLK = SUB * P              # 512 rows per block
NB = BC // BLK             # 8 blocks per core
KC = D_IN // P             # 8 contraction chunks for layer 1
HC = H // P                # 4 hidden chunks
SC = 4                     # sims spans of 1024 per sub-tile
SW = K // SC               # 1024 (two N=512 matmuls + one exp each)

_cached = None
last_results = None


def _build():
    nc = bacc.Bacc("TRN2", target_bir_lowering=False, debug=False)

    xT_d = nc.dram_tensor("inputsT", [D_IN, BC], F32, kind="ExternalInput")
    w1_d = nc.dram_tensor("W1", [D_IN, H], F32, kind="ExternalInput")
    b1_d = nc.dram_tensor("b1", [H], F32, kind="ExternalInput")
    w2_d = nc.dram_tensor("W2", [H, E], F32, kind="ExternalInput")
    b2_d = nc.dram_tensor("b2", [E], F32, kind="ExternalInput")
    pr_d = nc.dram_tensor("protos", [K, E], F32, kind="ExternalInput")
    pnT_d = nc.dram_tensor("pro_nT", [E, K], F32, kind="ExternalInput")
    w3_d = nc.dram_tensor("W3", [2 * E, E], F32, kind="ExternalInput")
    b3_d = nc.dram_tensor("b3", [E], F32, kind="ExternalInput")
    w4_d = nc.dram_tensor("W4", [E, E], F32, kind="ExternalInput")
    b4_d = nc.dram_tensor("b4", [E], F32, kind="ExternalInput")
    wh_d = nc.dram_tensor("Wh", [L, 2 * E], F32, kind="ExternalInput")
    bh_d = nc.dram_tensor("bh", [L], F32, kind="ExternalInput")

    probs_d = nc.dram_tensor("probs", [BC, K], F32, kind="ExternalOutput")
    abst_d = nc.dram_tensor("abst", [BC, E], F32, kind="ExternalOutput")
    hier_d = nc.dram_tensor("hier", [BC, L], F32, kind="ExternalOutput")

    with tile.TileContext(nc) as tc, ExitStack() as ctx:
        const = ctx.enter_context(tc.tile_pool(name="const", bufs=1))
        work = ctx.enter_context(tc.tile_pool(name="work", bufs=3))
        bigp = ctx.enter_context(tc.tile_pool(name="bigp", bufs=2))
        expp = ctx.enter_context(tc.tile_pool(name="expp", bufs=3))
        ps_h1 = ctx.enter_context(tc.tile_pool(name="ps_h1", bufs=2, space="PSUM"))
        ps_en = ctx.enter_context(tc.tile_pool(name="ps_en", bufs=1, space="PSUM"))
        ps_si = ctx.enter_context(tc.tile_pool(name="ps_si", bufs=2, space="PSUM"))
        ps_lt = ctx.enter_context(tc.tile_pool(name="ps_lt", bufs=1, space="PSUM"))

        # ---------------- prologue: constants (DMAs only) ----------------
        ident = const.tile([P, P], F32)
        make_identity(nc, ident[:])
        ones = const.tile([P, 1], F32)
        nc.vector.memset(ones[:], 1.0)
        nhalf = const.tile([P, SUB], F32)
        nc.vector.memset(nhalf[:], -0.5)

        w1sb = const.tile([P, KC * H], F32)       # block kc at [:, kc*512:...]
        for kc in range(KC):
            nc.sync.dma_start(
                w1sb[:, kc * H:(kc + 1) * H], w1_d[kc * P:(kc + 1) * P, :]
            )
        w2sb = const.tile([P, HC * E], F32)
        for hc in range(HC):
            nc.sync.dma_start(
                w2sb[:, hc * E:(hc + 1) * E], w2_d[hc * P:(hc + 1) * P, :]
            )
        b1c = const.tile([P, HC], F32)
        nc.sync.dma_start(b1c[:], b1_d.ap().rearrange("(c p) -> p c", p=P))
        b2c = const.tile([P, 1], F32)
        nc.sync.dma_start(b2c[:], b2_d.ap().rearrange("(c p) -> p c", p=P))

        # needed only from the sims stage on — emitted after the first
        # encoder DMAs so they don't delay the first H1 matmuls
        pro_nT = const.tile([P, K], F32)
        nc.sync.dma_start(pro_nT[:], pnT_d[:])
        w3sb = const.tile([P, 2 * E], F32)
        for c in range(2):
            nc.sync.dma_start(
                w3sb[:, c * E:(c + 1) * E], w3_d[c * P:(c + 1) * P, :]
            )
        w4sb = const.tile([P, E], F32)
        nc.sync.dma_start(w4sb[:], w4_d[:])
        whT = const.tile([P, 2 * L], F32)         # chunk c at [:, c*L:(c+1)*L]
        for c in range(2):
            nc.sync.dma_start(
                whT[:, c * L:(c + 1) * L],
                wh_d[:, c * P:(c + 1) * P].rearrange("l p -> p l"),
            )
        b3c = const.tile([P, 1], F32)
        nc.sync.dma_start(b3c[:], b3_d.ap().rearrange("(c p) -> p c", p=P))
        b4row = const.tile([1, E], F32)
        nc.sync.dma_start(b4row[:], b4_d.ap().rearrange("(o f) -> o f", o=1))
        b4b = const.tile([P, E], F32)
        nc.gpsimd.partition_broadcast(b4b[:], b4row[:])
        bhrow = const.tile([1, L], F32)
        nc.sync.dma_start(bhrow[:], bh_d.ap().rearrange("(o f) -> o f", o=1))
        bhb = const.tile([P, L], F32)
        nc.gpsimd.partition_broadcast(bhb[:], bhrow[:])

        def emit_late(e0_, s, sr0, bp):
            """abstraction + hierarchy for one sub-tile (runs one sub late
            so the PE never stalls on the gather's DMA semaphore)."""
            bpt_ps = ps_lt.tile([P, P], F32, tag="lt")
            nc.tensor.transpose(bpt_ps[:], bp[:], ident[:])
            bpts = work.tile([P, E], F32, tag="bpts")
            nc.scalar.copy(bpts[:], bpt_ps[:])

            a1_ps = ps_lt.tile([P, E], F32, tag="lt")
            nc.tensor.matmul(
                a1_ps[:], w3sb[:, :E], e0_[:, s * P:(s + 1) * P],
                start=True, stop=False,
            )
            nc.tensor.matmul(
                a1_ps[:], w3sb[:, E:], bpts[:], start=False, stop=True
            )
            a1 = work.tile([P, E], F32, tag="a1")
            nc.scalar.activation(a1[:], a1_ps[:], AF.Relu, bias=b3c[:, :1])
            ab_ps = ps_lt.tile([P, E], F32, tag="lt")
            nc.tensor.matmul(ab_ps[:], a1[:], w4sb[:], start=True, stop=True)
            ab = work.tile([P, E], F32, tag="ab")
            nc.vector.tensor_tensor(
                out=ab[:], in0=ab_ps[:], in1=b4b[:], op=OP.add
            )
            nc.sync.dma_start(abst_d[sr0:sr0 + P, :], ab[:])

            hr_ps = ps_lt.tile([P, L], F32, tag="lt")
            nc.tensor.matmul(
                hr_ps[:], e0_[:, s * P:(s + 1) * P], whT[:, :L],
                start=True, stop=False,
            )
            nc.tensor.matmul(
                hr_ps[:], bpts[:], whT[:, L:], start=False, stop=True
            )
            hz = work.tile([P, L], F32, tag="hz")
            nc.vector.tensor_tensor(
                out=hz[:], in0=hr_ps[:], in1=bhb[:], op=OP.add
            )
            he = work.tile([P, L], F32, tag="he")
            nc.scalar.activation(he[:], hz[:], AF.Exp, scale=-1.0)
            hd = work.tile([P, L], F32, tag="hd")
            nc.vector.tensor_scalar(
                out=hd[:], in0=he[:], scalar1=1.0, scalar2=None, op0=OP.add,
            )
            hs = work.tile([P, L], F32, tag="hs")
            nc.vector.reciprocal(hs[:], hd[:])
            nc.sync.dma_start(hier_d[sr0:sr0 + P, :], hs[:])

        pending = []

        # ---------------- main loop over 8 blocks of 512 rows ----------------
        for b in range(NB):
            r0 = b * BLK
            # x^T feature-major straight from DRAM (host pre-transposed)
            xt = bigp.tile([P, KC * BLK], F32, tag="xt")
            for kc in range(KC):
                nc.sync.dma_start(
                    xt[:, kc * BLK:(kc + 1) * BLK],
                    xT_d[kc * P:(kc + 1) * P, r0:r0 + BLK],
                )

            # H1^T = relu(W1^T x^T + b1): 4 chunks of [128h, 512b]
            h1 = bigp.tile([P, HC * BLK], F32, tag="h1")
            for hc in range(HC):
                h1_ps = ps_h1.tile([P, BLK], F32, tag="h1p")
                for kc in range(KC):
                    nc.tensor.matmul(
                        h1_ps[:],
                        w1sb[:, kc * H + hc * P: kc * H + (hc + 1) * P],
                        xt[:, kc * BLK:(kc + 1) * BLK],
                        start=(kc == 0), stop=(kc == KC - 1),
                    )
                nc.scalar.activation(
                    h1[:, hc * BLK:(hc + 1) * BLK], h1_ps[:],
                    AF.Relu, bias=b1c[:, hc:hc + 1],
                )

            # encoded^T [E, 512b]
            e0_ps = ps_en.tile([P, BLK], F32, tag="en")
            for hc in range(HC):
                nc.tensor.matmul(
                    e0_ps[:], w2sb[:, hc * E:(hc + 1) * E],
                    h1[:, hc * BLK:(hc + 1) * BLK],
                    start=(hc == 0), stop=(hc == HC - 1),
                )
            e0 = work.tile([P, BLK], F32, tag="e0")
            nc.scalar.activation(e0[:], e0_ps[:], AF.Identity, bias=b2c[:, :1])

            # per-row norms: [b,1] per sub via ones-matmul, packed [128, SUB]
            e0sq = work.tile([P, BLK], F32, tag="e0sq")
            nc.scalar.activation(e0sq[:], e0[:], AF.Square)
            ns_ps = ps_en.tile([P, SUB], F32, tag="en")
            for s in range(SUB):
                nc.tensor.matmul(
                    ns_ps[:, s:s + 1], e0sq[:, s * P:(s + 1) * P], ones[:],
                    start=True, stop=True,
                )
            ns = work.tile([P, SUB], F32, tag="ns")
            nc.vector.tensor_copy(ns[:], ns_ps[:])
            # scale = 10 * min(pow(s, -0.5), 1e8)
            inv = work.tile([P, SUB], F32, tag="inv")
            nc.gpsimd.tensor_tensor(out=inv[:], in0=ns[:], in1=nhalf[:], op=OP.pow)
            scl = work.tile([P, SUB], F32, tag="scl")
            nc.vector.tensor_scalar(
                out=scl[:], in0=inv[:], scalar1=1e8, scalar2=10.0,
                op0=OP.min, op1=OP.mult,
            )

            for s in range(SUB):
                sr0 = r0 + s * P
                # sims -> exp(scale * sims); accum_out -> span sums
                ex = expp.tile([P, K], F32, tag="ex")
                s4 = work.tile([P, SC], F32, tag="s4")
                for c in range(SC):
                    si_ps = ps_si.tile([P, SW], F32, tag="si")
                    for half in range(2):
                        nc.tensor.matmul(
                            si_ps[:, half * 512:(half + 1) * 512],
                            e0[:, s * P:(s + 1) * P],
                            pro_nT[:, c * SW + half * 512: c * SW + (half + 1) * 512],
                            start=True, stop=True,
                        )
                    nc.scalar.activation(
                        ex[:, c * SW:(c + 1) * SW], si_ps[:], AF.Exp,
                        scale=scl[:, s:s + 1], accum_out=s4[:, c:c + 1],
                    )
                ssum = work.tile([P, 1], F32, tag="ssum")
                nc.vector.reduce_sum(
                    out=ssum[:], in_=s4[:], axis=mybir.AxisListType.X
                )
                rs = work.tile([P, 1], F32, tag="rs")
                nc.vector.reciprocal(rs[:], ssum[:])

                # argmax over exp values (same ordering as probs)
                mx8 = work.tile([P, 8], F32, tag="mx8")
                nc.vector.max(out=mx8[:], in_=ex[:])
                mi8 = work.tile([P, 8], U32, tag="mi8")
                nc.vector.max_index(out=mi8[:], in_max=mx8[:], in_values=ex[:])
                bp = work.tile([P, E], F32, tag="bp")
                nc.gpsimd.indirect_dma_start(
                    out=bp[:], out_offset=None, in_=pr_d[:],
                    in_offset=bass.IndirectOffsetOnAxis(ap=mi8[:, :1], axis=0),
                )

                # probs = ex / sum  (in place), then DMA out
                nc.vector.tensor_scalar(
                    out=ex[:], in0=ex[:], scalar1=rs[:, :1], scalar2=None,
                    op0=OP.mult,
                )
                nc.sync.dma_start(probs_d[sr0:sr0 + P, :], ex[:])

                # late chain runs TWO subs late so the PE never stalls
                # on the gather's DMA semaphore
                pending.append((e0, s, sr0, bp))
                if len(pending) > 2:
                    emit_late(*pending.pop(0))

        for args in pending:
            emit_late(*args)

    nc.compile()
    return nc


def _prep_in_maps(inputs):
    full = {k: np.ascontiguousarray(np.asarray(v, dtype=np.float32))
            for k, v in inputs.items()}
    x = full.pop("inputs")
    protos = full["protos"]
    norms = np.maximum(
        np.linalg.norm(protos.astype(np.float64), axis=1, keepdims=True), 1e-8
    )
    full["pro_nT"] = np.ascontiguousarray(
        (protos.astype(np.float64) / norms).T.astype(np.float32)
    )
    xT = x.T  # [D_IN, B]
    in_maps = []
    for c in range(N_CORES):
        m = dict(full)
        m["inputsT"] = np.ascontiguousarray(xT[:, c * BC:(c + 1) * BC])
        in_maps.append(m)
    return in_maps


def kernel(**inputs):
    global _cached, last_results
    if _cached is None:
        _cached = _build()
    res = run_bass_kernel_spmd(_cached, _prep_in_maps(inputs),
                               list(range(N_CORES)))
    last_results = res
    probs = np.concatenate([r["probs"] for r in res.results], axis=0)
    abst = np.concatenate([r["abst"] for r in res.results], axis=0)
    hier = np.concatenate([r["hier"] for r in res.results], axis=0)
    return probs, abst, hier


def run_traced(inputs):
    """Profiled run (test-harness helper; requires the axon NTFF hook)."""
    global _cached
    if _cached is None:
        _cached = _build()
    return run_bass_kernel_spmd(_cached, _prep_in_maps(inputs),
                                list(range(N_CORES)), trace=True)


# revision 10
# speedup vs baseline: 1.3238x; 1.0321x over previous
"""Trainium2 Bass kernel for nn_BidirectionalNeuralSymbolic (vq_codebook).

Model (per batch row x of dim 1024):
  encoded = relu(x @ W1 + b1) @ W2 + b2                      # [E=128]
  sims    = cos(encoded, protos[k]) for k in 4096            # [K]
  concept_probs = softmax(10 * sims)                         # output 1 [B,K]
  best = argmax(concept_probs); bp = protos[best]            # gather
  ai = [encoded, bp]                                         # [256]
  abstraction = relu(ai @ W3 + b3) @ W4 + b4                 # output 2 [B,E]
  hierarchy   = sigmoid(ai @ Wh.T + bh)                      # output 3 [B,L]

Sharding: pure data-parallel over batch across 8 NeuronCores; all params
replicated. Each core handles 4096 rows as 8 blocks of 512 (4 sub-tiles
of 128) so the heavy matmuls run with a 512-wide moving operand.

Host-side prep inside kernel(): per-core batch slices are pre-transposed
([D_IN, 4096], the layout the feature-major matmuls consume) and the
codebook is pre-normalized/transposed once ([E, K]); raw protos stay in
DRAM for the argmax gather. All on-device math is fp32 — the top-2 sims
gap is as small as 9e-7 on these inputs, so any reduced-precision matmul
would flip argmax rows and corrupt `abstraction`.

Engine plan per 512-row block:
  PE : H1 (32 mm N=512), encoded^T (4 mm N=512), row-norm ones-matmuls,
       sims (32 mm N=512), best-proto transposes, abstraction/hierarchy.
  ACT: PSUM evacuations fused with bias/ReLU; exp of sims over [128,1024]
       PSUM spans with the per-row 10/||enc|| as the per-partition scale
       and accum_out producing softmax denominators for free. Only
       EXP-set table functions are used -> single ACT table load.
  DVE: max8 + max_index argmax (first-index ties = jnp.argmax), smalls.
  GPS: rsqrt via pow(s,-0.5) (Q7 vpowf), softmax divide (tensor_scalar
       by 1/sum, exact), indirect-DMA gather of argmax protos rows.
PSUM banks: h1 2 + e0/ns 1 + sims 4 + late 1 = 8.
"""

import numpy as np
from contextlib import ExitStack

import concourse.bass as bass
import concourse.mybir as mybir
import concourse.tile as tile
from concourse import bacc
from concourse.bass_utils import run_bass_kernel_spmd
from concourse.masks import make_identity

F32 = mybir.dt.float32
U32 = mybir.dt.uint32
AF = mybir.ActivationFunctionType
OP = mybir.AluOpType

B, D_IN, K, E, L, H = 32768, 1024, 4096, 128, 5, 512
N_CORES = 8
BC = B // N_CORES          # 4096 rows per core
P = 128
SUB = 4                    # 128-row sub-tiles per block
BLK = SUB * P              # 512 rows per block
NB = BC // BLK             # 8 blocks per core
KC = D_IN // P             # 8 contraction chunks for layer 1
HC = H // P                # 4 hidden chunks
SC = 4                     # sims spans of 1024 per sub-tile
SW = K // SC               # 1024 (two N=512 matmuls + one exp each)

_cached = None
last_results = None


def _build():
    nc = bacc.Bacc("TRN2", target_bir_lowering=False, debug=False)

    xT_d = nc.dram_tensor("inputsT", [D_IN, BC], F32, kind="ExternalInput")
    w1_d = nc.dram_tensor("W1", [D_IN, H], F32, kind="ExternalInput")
    b1_d = nc.dram_tensor("b1", [H], F32, kind="ExternalInput")
    w2_d = nc.dram_tensor("W2", [H, E], F32, kind="ExternalInput")
    b2_d = nc.dram_tensor("b2", [E], F32, kind="ExternalInput")
    pr_d = nc.dram_tensor("protos", [K, E], F32, kind="ExternalInput")
    pnT_d = nc.dram_tensor("pro_nT", [E, K], F32, kind="ExternalInput")
    w3_d = nc.dram_tensor("W3", [2 * E, E], F32, kind="ExternalInput")
    b3_d = nc.dram_tensor("b3", [E], F32, kind="ExternalInput")
    w4_d = nc.dram_tensor("W4", [E, E], F32, kind="ExternalInput")
    b4_d = nc.dram_tensor("b4", [E], F32, kind="ExternalInput")
    wh_d = nc.dram_tensor("Wh", [L, 2 * E], F32, kind="ExternalInput")
    bh_d = nc.dram_tensor("bh", [L], F32, kind="ExternalInput")

    probs_d = nc.dram_tensor("probs", [BC, K], F32, kind="ExternalOutput")
    abst_d = nc.dram_tensor("abst", [BC, E], F32, kind="ExternalOutput")
    hier_d = nc.dram_tensor("hier", [BC, L], F32, kind="ExternalOutput")

    with tile.TileContext(nc) as tc, ExitStack() as ctx:
        const = ctx.enter_context(tc.tile_pool(name="const", bufs=1))
        work = ctx.enter_context(tc.tile_pool(name="work", bufs=3))
        bigp = ctx.enter_context(tc.tile_pool(name="bigp", bufs=2))
        expp = ctx.enter_context(tc.tile_pool(name="expp", bufs=3))
        ps_h1 = ctx.enter_context(tc.tile_pool(name="ps_h1", bufs=2, space="PSUM"))
        ps_en = ctx.enter_context(tc.tile_pool(name="ps_en", bufs=1, space="PSUM"))
        ps_si = ctx.enter_context(tc.tile_pool(name="ps_si", bufs=2, space="PSUM"))
        ps_lt = ctx.enter_context(tc.tile_pool(name="ps_lt", bufs=1, space="PSUM"))

        # ---------------- prologue: constants (DMAs only) ----------------
        ident = const.tile([P, P], F32)
        make_identity(nc, ident[:])
        ones = const.tile([P, 1], F32)
        nc.vector.memset(ones[:], 1.0)
        nhalf = const.tile([P, SUB], F32)
        nc.vector.memset(nhalf[:], -0.5)

        w1sb = const.tile([P, KC * H], F32)       # block kc at [:, kc*512:...]
        w2sb = const.tile([P, HC * E], F32)
        b1c = const.tile([P, HC], F32)
        b2c = const.tile([P, 1], F32)
        pro_nT = const.tile([P, K], F32)
        w3sb = const.tile([P, 2 * E], F32)
        w4sb = const.tile([P, E], F32)
        whT = const.tile([P, 2 * L], F32)         # chunk c at [:, c*L:(c+1)*L]
        b3c = const.tile([P, 1], F32)
        b4row = const.tile([1, E], F32)
        b4b = const.tile([P, E], F32)
        bhrow = const.tile([1, L], F32)
        bhb = const.tile([P, L], F32)

        def emit_early_consts():
            # only what the very first H1 accumulation chain touches goes
            # first; everything else is emitted after block 0's xt DMAs
            for hc in range(HC):
                nc.sync.dma_start(
                    w2sb[:, hc * E:(hc + 1) * E], w2_d[hc * P:(hc + 1) * P, :]
                )
            nc.sync.dma_start(b1c[:], b1_d.ap().rearrange("(c p) -> p c", p=P))
            nc.sync.dma_start(b2c[:], b2_d.ap().rearrange("(c p) -> p c", p=P))

        def emit_late_consts():
            nc.sync.dma_start(pro_nT[:], pnT_d[:])
            for c in range(2):
                nc.sync.dma_start(
                    w3sb[:, c * E:(c + 1) * E], w3_d[c * P:(c + 1) * P, :]
                )
            nc.sync.dma_start(w4sb[:], w4_d[:])
            for c in range(2):
                nc.sync.dma_start(
                    whT[:, c * L:(c + 1) * L],
                    wh_d[:, c * P:(c + 1) * P].rearrange("l p -> p l"),
                )
            nc.sync.dma_start(b3c[:], b3_d.ap().rearrange("(c p) -> p c", p=P))
            nc.sync.dma_start(
                b4row[:], b4_d.ap().rearrange("(o f) -> o f", o=1)
            )
            nc.gpsimd.partition_broadcast(b4b[:], b4row[:])
            nc.sync.dma_start(
                bhrow[:], bh_d.ap().rearrange("(o f) -> o f", o=1)
            )
            nc.gpsimd.partition_broadcast(bhb[:], bhrow[:])

        def emit_late(e0_, s, sr0, bp):
            """abstraction + hierarchy for one sub-tile (runs one sub late
            so the PE never stalls on the gather's DMA semaphore)."""
            bpt_ps = ps_lt.tile([P, P], F32, tag="lt")
            nc.tensor.transpose(bpt_ps[:], bp[:], ident[:])
            bpts = work.tile([P, E], F32, tag="bpts")
            nc.scalar.copy(bpts[:], bpt_ps[:])

            a1_ps = ps_lt.tile([P, E], F32, tag="lt")
            nc.tensor.matmul(
                a1_ps[:], w3sb[:, :E], e0_[:, s * P:(s + 1) * P],
                start=True, stop=False,
            )
            nc.tensor.matmul(
                a1_ps[:], w3sb[:, E:], bpts[:], start=False, stop=True
            )
            a1 = work.tile([P, E], F32, tag="a1")
            nc.scalar.activation(a1[:], a1_ps[:], AF.Relu, bias=b3c[:, :1])
            ab_ps = ps_lt.tile([P, E], F32, tag="lt")
            nc.tensor.matmul(ab_ps[:], a1[:], w4sb[:], start=True, stop=True)
            ab = work.tile([P, E], F32, tag="ab")
            nc.vector.tensor_tensor(
                out=ab[:], in0=ab_ps[:], in1=b4b[:], op=OP.add
            )
            nc.sync.dma_start(abst_d[sr0:sr0 + P, :], ab[:])

            hr_ps = ps_lt.tile([P, L], F32, tag="lt")
            nc.tensor.matmul(
                hr_ps[:], e0_[:, s * P:(s + 1) * P], whT[:, :L],
                start=True, stop=False,
            )
            nc.tensor.matmul(
                hr_ps[:], bpts[:], whT[:, L:], start=False, stop=True
            )
            hz = work.tile([P, L], F32, tag="hz")
            nc.vector.tensor_tensor(
                out=hz[:], in0=hr_ps[:], in1=bhb[:], op=OP.add
            )
            he = work.tile([P, L], F32, tag="he")
            nc.scalar.activation(he[:], hz[:], AF.Exp, scale=-1.0)
            hd = work.tile([P, L], F32, tag="hd")
            nc.vector.tensor_scalar(
                out=hd[:], in0=he[:], scalar1=1.0, scalar2=None, op0=OP.add,
            )
            hs = work.tile([P, L], F32, tag="hs")
            nc.vector.reciprocal(hs[:], hd[:])
            nc.sync.dma_start(hier_d[sr0:sr0 + P, :], hs[:])

        pending = []

        # ---------------- main loop over 8 blocks of 512 rows ----------------
        for b in range(NB):
            r0 = b * BLK
            # x^T feature-major straight from DRAM (host pre-transposed)
            xt = bigp.tile([P, KC * BLK], F32, tag="xt")
            for kc in range(KC):
                if b == 0:
                    nc.sync.dma_start(
                        w1sb[:, kc * H:(kc + 1) * H],
                        w1_d[kc * P:(kc + 1) * P, :],
                    )
                nc.sync.dma_start(
                    xt[:, kc * BLK:(kc + 1) * BLK],
                    xT_d[kc * P:(kc + 1) * P, r0:r0 + BLK],
                )
            if b == 0:
                emit_early_consts()
                emit_late_consts()

            # H1^T = relu(W1^T x^T + b1): 4 chunks of [128h, 512b]
            h1 = bigp.tile([P, HC * BLK], F32, tag="h1")
            for hc in range(HC):
                h1_ps = ps_h1.tile([P, BLK], F32, tag="h1p")
                for kc in range(KC):
                    nc.tensor.matmul(
                        h1_ps[:],
                        w1sb[:, kc * H + hc * P: kc * H + (hc + 1) * P],
                        xt[:, kc * BLK:(kc + 1) * BLK],
                        start=(kc == 0), stop=(kc == KC - 1),
                    )
                nc.scalar.activation(
                    h1[:, hc * BLK:(hc + 1) * BLK], h1_ps[:],
                    AF.Relu, bias=b1c[:, hc:hc + 1],
                )

            # encoded^T [E, 512b]
            e0_ps = ps_en.tile([P, BLK], F32, tag="en")
            for hc in range(HC):
                nc.tensor.matmul(
                    e0_ps[:], w2sb[:, hc * E:(hc + 1) * E],
                    h1[:, hc * BLK:(hc + 1) * BLK],
                    start=(hc == 0), stop=(hc == HC - 1),
                )
            e0 = work.tile([P, BLK], F32, tag="e0")
            nc.scalar.activation(e0[:], e0_ps[:], AF.Identity, bias=b2c[:, :1])

            # per-row norms: [b,1] per sub via ones-matmul, packed [128, SUB]
            e0sq = work.tile([P, BLK], F32, tag="e0sq")
            nc.scalar.activation(e0sq[:], e0[:], AF.Square)
            ns_ps = ps_en.tile([P, SUB], F32, tag="en")
            for s in range(SUB):
                nc.tensor.matmul(
                    ns_ps[:, s:s + 1], e0sq[:, s * P:(s + 1) * P], ones[:],
                    start=True, stop=True,
                )
            ns = work.tile([P, SUB], F32, tag="ns")
            nc.vector.tensor_copy(ns[:], ns_ps[:])
            # scale = 10 * min(pow(s, -0.5), 1e8)
            inv = work.tile([P, SUB], F32, tag="inv")
            nc.gpsimd.tensor_tensor(out=inv[:], in0=ns[:], in1=nhalf[:], op=OP.pow)
            scl = work.tile([P, SUB], F32, tag="scl")
            nc.vector.tensor_scalar(
                out=scl[:], in0=inv[:], scalar1=1e8, scalar2=10.0,
                op0=OP.min, op1=OP.mult,
            )

            for s in range(SUB):
                sr0 = r0 + s * P
                # sims -> exp(scale * sims); accum_out -> span sums
                ex = expp.tile([P, K], F32, tag="ex")
                s4 = work.tile([P, SC], F32, tag="s4")
                for c in range(SC):
                    si_ps = ps_si.tile([P, SW], F32, tag="si")
                    for half in range(2):
                        nc.tensor.matmul(
                            si_ps[:, half * 512:(half + 1) * 512],
                            e0[:, s * P:(s + 1) * P],
                            pro_nT[:, c * SW + half * 512: c * SW + (half + 1) * 512],
                            start=True, stop=True,
                        )
                    nc.scalar.activation(
                        ex[:, c * SW:(c + 1) * SW], si_ps[:], AF.Exp,
                        scale=scl[:, s:s + 1], accum_out=s4[:, c:c + 1],
                    )
                ssum = work.tile([P, 1], F32, tag="ssum")
                nc.vector.reduce_sum(
                    out=ssum[:], in_=s4[:], axis=mybir.AxisListType.X
                )
                rs = work.tile([P, 1], F32, tag="rs")
                nc.vector.reciprocal(rs[:], ssum[:])

                # argmax over exp values (same ordering as probs)
                mx8 = work.tile([P, 8], F32, tag="mx8")
                nc.vector.max(out=mx8[:], in_=ex[:])
                mi8 = work.tile([P, 8], U32, tag="mi8")
                nc.vector.max_index(out=mi8[:], in_max=mx8[:], in_values=ex[:])
                bp = work.tile([P, E], F32, tag="bp")
                nc.gpsimd.indirect_dma_start(
                    out=bp[:], out_offset=None, in_=pr_d[:],
                    in_offset=bass.IndirectOffsetOnAxis(ap=mi8[:, :1], axis=0),
                )

                # probs = ex / sum  (in place), then DMA out
                nc.vector.tensor_scalar(
                    out=ex[:], in0=ex[:], scalar1=rs[:, :1], scalar2=None,
                    op0=OP.mult,
                )
                nc.sync.dma_start(probs_d[sr0:sr0 + P, :], ex[:])

                # late chain runs TWO subs late so the PE never stalls
                # on the gather's DMA semaphore
                pending.append((e0, s, sr0, bp))
                if len(pending) > 2:
                    emit_late(*pending.pop(0))

        for args in pending:
            emit_late(*args)

    nc.compile()
    return nc


def _prep_in_maps(inputs):
    full = {k: np.ascontiguousarray(np.asarray(v, dtype=np.float32))
            for k, v in inputs.items()}
    x = full.pop("inputs")
    protos = full["protos"]
    norms = np.maximum(
        np.linalg.norm(protos.astype(np.float64), axis=1, keepdims=True), 1e-8
    )
    full["pro_nT"] = np.ascontiguousarray(
        (protos.astype(np.float64) / norms).T.astype(np.float32)
    )
    xT = x.T  # [D_IN, B]
    in_maps = []
    for c in range(N_CORES):
        m = dict(full)
        m["inputsT"] = np.ascontiguousarray(xT[:, c * BC:(c + 1) * BC])
        in_maps.append(m)
    return in_maps


def kernel(**inputs):
    global _cached, last_results
    if _cached is None:
        _cached = _build()
    res = run_bass_kernel_spmd(_cached, _prep_in_maps(inputs),
                               list(range(N_CORES)))
    last_results = res
    probs = np.concatenate([r["probs"] for r in res.results], axis=0)
    abst = np.concatenate([r["abst"] for r in res.results], axis=0)
    hier = np.concatenate([r["hier"] for r in res.results], axis=0)
    return probs, abst, hier


def run_traced(inputs):
    """Profiled run (test-harness helper; requires the axon NTFF hook)."""
    global _cached
    if _cached is None:
        _cached = _build()
    return run_bass_kernel_spmd(_cached, _prep_in_maps(inputs),
                                list(range(N_CORES)), trace=True)


# revision 11
# speedup vs baseline: 1.3262x; 1.0018x over previous
"""Trainium2 Bass kernel for nn_BidirectionalNeuralSymbolic (vq_codebook).

Model (per batch row x of dim 1024):
  encoded = relu(x @ W1 + b1) @ W2 + b2                      # [E=128]
  sims    = cos(encoded, protos[k]) for k in 4096            # [K]
  concept_probs = softmax(10 * sims)                         # output 1 [B,K]
  best = argmax(concept_probs); bp = protos[best]            # gather
  ai = [encoded, bp]                                         # [256]
  abstraction = relu(ai @ W3 + b3) @ W4 + b4                 # output 2 [B,E]
  hierarchy   = sigmoid(ai @ Wh.T + bh)                      # output 3 [B,L]

Sharding: pure data-parallel over batch across 8 NeuronCores; all params
replicated. Each core handles 4096 rows as 8 blocks of 512 (4 sub-tiles
of 128) so the heavy matmuls run with a 512-wide moving operand.

Host-side prep inside kernel(): per-core batch slices are pre-transposed
([D_IN, 4096], the layout the feature-major matmuls consume) and the
codebook is pre-normalized/transposed once ([E, K]); raw protos stay in
DRAM for the argmax gather. All on-device math is fp32 — the top-2 sims
gap is as small as 9e-7 on these inputs, so any reduced-precision matmul
would flip argmax rows and corrupt `abstraction`.

Engine plan per 512-row block:
  PE : H1 (32 mm N=512), encoded^T (4 mm N=512), row-norm ones-matmuls,
       sims (32 mm N=512), best-proto transposes, abstraction/hierarchy.
  ACT: PSUM evacuations fused with bias/ReLU; exp of sims over [128,1024]
       PSUM spans with the per-row 10/||enc|| as the per-partition scale
       and accum_out producing softmax denominators for free. Only
       EXP-set table functions are used -> single ACT table load.
  DVE: max8 + max_index argmax (first-index ties = jnp.argmax), smalls.
  GPS: rsqrt via pow(s,-0.5) (Q7 vpowf), softmax divide (tensor_scalar
       by 1/sum, exact), indirect-DMA gather of argmax protos rows.
PSUM banks: h1 2 + e0/ns 1 + sims 4 + late 1 = 8.
"""

import numpy as np
from contextlib import ExitStack

import concourse.bass as bass
import concourse.mybir as mybir
import concourse.tile as tile
from concourse import bacc
from concourse.bass_utils import run_bass_kernel_spmd
from concourse.masks import make_identity

F32 = mybir.dt.float32
U32 = mybir.dt.uint32
AF = mybir.ActivationFunctionType
OP = mybir.AluOpType

B, D_IN, K, E, L, H = 32768, 1024, 4096, 128, 5, 512
N_CORES = 8
BC = B // N_CORES          # 4096 rows per core
P = 128
SUB = 4                    # 128-row sub-tiles per block
BLK = SUB * P              # 512 rows per block
NB = BC // BLK             # 8 blocks per core
KC = D_IN // P             # 8 contraction chunks for layer 1
HC = H // P                # 4 hidden chunks
SC = 4                     # sims spans of 1024 per sub-tile
SW = K // SC               # 1024 (two N=512 matmuls + one exp each)

_cached = None
last_results = None


def _build():
    nc = bacc.Bacc("TRN2", target_bir_lowering=False, debug=False)

    xT_d = nc.dram_tensor("inputsT", [D_IN, BC], F32, kind="ExternalInput")
    w1_d = nc.dram_tensor("W1", [D_IN, H], F32, kind="ExternalInput")
    b1_d = nc.dram_tensor("b1", [H], F32, kind="ExternalInput")
    w2_d = nc.dram_tensor("W2", [H, E], F32, kind="ExternalInput")
    b2_d = nc.dram_tensor("b2", [E], F32, kind="ExternalInput")
    pr_d = nc.dram_tensor("protos", [K, E], F32, kind="ExternalInput")
    pnT_d = nc.dram_tensor("pro_nT", [E, K], F32, kind="ExternalInput")
    w3_d = nc.dram_tensor("W3", [2 * E, E], F32, kind="ExternalInput")
    b3_d = nc.dram_tensor("b3", [E], F32, kind="ExternalInput")
    w4_d = nc.dram_tensor("W4", [E, E], F32, kind="ExternalInput")
    b4_d = nc.dram_tensor("b4", [E], F32, kind="ExternalInput")
    wh_d = nc.dram_tensor("Wh", [L, 2 * E], F32, kind="ExternalInput")
    bh_d = nc.dram_tensor("bh", [L], F32, kind="ExternalInput")

    probs_d = nc.dram_tensor("probs", [BC, K], F32, kind="ExternalOutput")
    abst_d = nc.dram_tensor("abst", [BC, E], F32, kind="ExternalOutput")
    hier_d = nc.dram_tensor("hier", [BC, L], F32, kind="ExternalOutput")

    with tile.TileContext(nc) as tc, ExitStack() as ctx:
        const = ctx.enter_context(tc.tile_pool(name="const", bufs=1))
        work = ctx.enter_context(tc.tile_pool(name="work", bufs=3))
        bigp = ctx.enter_context(tc.tile_pool(name="bigp", bufs=2))
        expp = ctx.enter_context(tc.tile_pool(name="expp", bufs=3))
        ps_h1 = ctx.enter_context(tc.tile_pool(name="ps_h1", bufs=2, space="PSUM"))
        ps_en = ctx.enter_context(tc.tile_pool(name="ps_en", bufs=1, space="PSUM"))
        ps_si = ctx.enter_context(tc.tile_pool(name="ps_si", bufs=2, space="PSUM"))
        ps_lt = ctx.enter_context(tc.tile_pool(name="ps_lt", bufs=1, space="PSUM"))

        # ---------------- prologue: constants (DMAs only) ----------------
        ident = const.tile([P, P], F32)
        make_identity(nc, ident[:])
        ones = const.tile([P, 1], F32)
        nc.vector.memset(ones[:], 1.0)
        nhalf = const.tile([P, SUB], F32)
        nc.vector.memset(nhalf[:], -0.5)

        w1sb = const.tile([P, KC * H], F32)       # block kc at [:, kc*512:...]
        w2sb = const.tile([P, HC * E], F32)
        b1c = const.tile([P, HC], F32)
        b2c = const.tile([P, 1], F32)
        pro_nT = const.tile([P, K], F32)
        w3sb = const.tile([P, 2 * E], F32)
        w4sb = const.tile([P, E], F32)
        whT = const.tile([P, 2 * L], F32)         # chunk c at [:, c*L:(c+1)*L]
        b3c = const.tile([P, 1], F32)
        b4row = const.tile([1, E], F32)
        b4b = const.tile([P, E], F32)
        bhrow = const.tile([1, L], F32)
        bhb = const.tile([P, L], F32)

        def emit_early_consts():
            # only what the very first H1 accumulation chain touches goes
            # first; everything else is emitted after block 0's xt DMAs
            for hc in range(HC):
                nc.sync.dma_start(
                    w2sb[:, hc * E:(hc + 1) * E], w2_d[hc * P:(hc + 1) * P, :]
                )
            nc.sync.dma_start(b1c[:], b1_d.ap().rearrange("(c p) -> p c", p=P))
            nc.sync.dma_start(b2c[:], b2_d.ap().rearrange("(c p) -> p c", p=P))

        def emit_late_consts():
            nc.sync.dma_start(pro_nT[:], pnT_d[:])
            for c in range(2):
                nc.sync.dma_start(
                    w3sb[:, c * E:(c + 1) * E], w3_d[c * P:(c + 1) * P, :]
                )
            nc.sync.dma_start(w4sb[:], w4_d[:])
            for c in range(2):
                nc.sync.dma_start(
                    whT[:, c * L:(c + 1) * L],
                    wh_d[:, c * P:(c + 1) * P].rearrange("l p -> p l"),
                )
            nc.sync.dma_start(b3c[:], b3_d.ap().rearrange("(c p) -> p c", p=P))
            nc.sync.dma_start(
                b4row[:], b4_d.ap().rearrange("(o f) -> o f", o=1)
            )
            nc.gpsimd.partition_broadcast(b4b[:], b4row[:])
            nc.sync.dma_start(
                bhrow[:], bh_d.ap().rearrange("(o f) -> o f", o=1)
            )
            nc.gpsimd.partition_broadcast(bhb[:], bhrow[:])

        def emit_late(e0_, s, sr0, bp):
            """abstraction + hierarchy for one sub-tile (runs one sub late
            so the PE never stalls on the gather's DMA semaphore)."""
            bpt_ps = ps_lt.tile([P, P], F32, tag="lt")
            nc.tensor.transpose(bpt_ps[:], bp[:], ident[:])
            bpts = work.tile([P, E], F32, tag="bpts")
            nc.scalar.copy(bpts[:], bpt_ps[:])

            a1_ps = ps_lt.tile([P, E], F32, tag="lt")
            nc.tensor.matmul(
                a1_ps[:], w3sb[:, :E], e0_[:, s * P:(s + 1) * P],
                start=True, stop=False,
            )
            nc.tensor.matmul(
                a1_ps[:], w3sb[:, E:], bpts[:], start=False, stop=True
            )
            a1 = work.tile([P, E], F32, tag="a1")
            nc.scalar.activation(a1[:], a1_ps[:], AF.Relu, bias=b3c[:, :1])
            ab_ps = ps_lt.tile([P, E], F32, tag="lt")
            nc.tensor.matmul(ab_ps[:], a1[:], w4sb[:], start=True, stop=True)
            ab = work.tile([P, E], F32, tag="ab")
            nc.vector.tensor_tensor(
                out=ab[:], in0=ab_ps[:], in1=b4b[:], op=OP.add
            )
            nc.sync.dma_start(abst_d[sr0:sr0 + P, :], ab[:])

            hr_ps = ps_lt.tile([P, L], F32, tag="lt")
            nc.tensor.matmul(
                hr_ps[:], e0_[:, s * P:(s + 1) * P], whT[:, :L],
                start=True, stop=False,
            )
            nc.tensor.matmul(
                hr_ps[:], bpts[:], whT[:, L:], start=False, stop=True
            )
            hz = work.tile([P, L], F32, tag="hz")
            nc.vector.tensor_tensor(
                out=hz[:], in0=hr_ps[:], in1=bhb[:], op=OP.add
            )
            he = work.tile([P, L], F32, tag="he")
            nc.scalar.activation(he[:], hz[:], AF.Exp, scale=-1.0)
            hd = work.tile([P, L], F32, tag="hd")
            nc.vector.tensor_scalar(
                out=hd[:], in0=he[:], scalar1=1.0, scalar2=None, op0=OP.add,
            )
            hs = work.tile([P, L], F32, tag="hs")
            nc.vector.reciprocal(hs[:], hd[:])
            nc.sync.dma_start(hier_d[sr0:sr0 + P, :], hs[:])

        pending = []

        # ---------------- main loop over 8 blocks of 512 rows ----------------
        for b in range(NB):
            r0 = b * BLK
            # x^T feature-major straight from DRAM (host pre-transposed)
            xt = bigp.tile([P, KC * BLK], F32, tag="xt")
            for kc in range(KC):
                if b == 0:
                    nc.sync.dma_start(
                        w1sb[:, kc * H:(kc + 1) * H],
                        w1_d[kc * P:(kc + 1) * P, :],
                    )
                nc.sync.dma_start(
                    xt[:, kc * BLK:(kc + 1) * BLK],
                    xT_d[kc * P:(kc + 1) * P, r0:r0 + BLK],
                )
            if b == 0:
                emit_early_consts()
                emit_late_consts()

            # H1^T = relu(W1^T x^T + b1): 4 chunks of [128h, 512b]
            h1 = bigp.tile([P, HC * BLK], F32, tag="h1")
            for hc in range(HC):
                h1_ps = ps_h1.tile([P, BLK], F32, tag="h1p")
                for kc in range(KC):
                    nc.tensor.matmul(
                        h1_ps[:],
                        w1sb[:, kc * H + hc * P: kc * H + (hc + 1) * P],
                        xt[:, kc * BLK:(kc + 1) * BLK],
                        start=(kc == 0), stop=(kc == KC - 1),
                    )
                nc.scalar.activation(
                    h1[:, hc * BLK:(hc + 1) * BLK], h1_ps[:],
                    AF.Relu, bias=b1c[:, hc:hc + 1],
                )

            # encoded^T [E, 512b]
            e0_ps = ps_en.tile([P, BLK], F32, tag="en")
            for hc in range(HC):
                nc.tensor.matmul(
                    e0_ps[:], w2sb[:, hc * E:(hc + 1) * E],
                    h1[:, hc * BLK:(hc + 1) * BLK],
                    start=(hc == 0), stop=(hc == HC - 1),
                )
            e0 = work.tile([P, BLK], F32, tag="e0")
            nc.scalar.activation(e0[:], e0_ps[:], AF.Identity, bias=b2c[:, :1])

            # per-row norms: [b,1] per sub via ones-matmul, packed [128, SUB]
            e0sq = work.tile([P, BLK], F32, tag="e0sq")
            nc.scalar.activation(e0sq[:], e0[:], AF.Square)
            ns_ps = ps_en.tile([P, SUB], F32, tag="en")
            for s in range(SUB):
                nc.tensor.matmul(
                    ns_ps[:, s:s + 1], e0sq[:, s * P:(s + 1) * P], ones[:],
                    start=True, stop=True,
                )
            ns = work.tile([P, SUB], F32, tag="ns")
            nc.vector.tensor_copy(ns[:], ns_ps[:])
            # scale = 10 * min(pow(s, -0.5), 1e8)
            inv = work.tile([P, SUB], F32, tag="inv")
            nc.gpsimd.tensor_tensor(out=inv[:], in0=ns[:], in1=nhalf[:], op=OP.pow)
            scl = work.tile([P, SUB], F32, tag="scl")
            nc.vector.tensor_scalar(
                out=scl[:], in0=inv[:], scalar1=1e8, scalar2=10.0,
                op0=OP.min, op1=OP.mult,
            )

            for s in range(SUB):
                sr0 = r0 + s * P
                # sims -> exp(scale * sims); accum_out -> span sums
                ex = expp.tile([P, K], F32, tag="ex")
                s4 = work.tile([P, SC], F32, tag="s4")
                for c in range(SC):
                    si_ps = ps_si.tile([P, SW], F32, tag="si")
                    for half in range(2):
                        nc.tensor.matmul(
                            si_ps[:, half * 512:(half + 1) * 512],
                            e0[:, s * P:(s + 1) * P],
                            pro_nT[:, c * SW + half * 512: c * SW + (half + 1) * 512],
                            start=True, stop=True,
                        )
                    nc.scalar.activation(
                        ex[:, c * SW:(c + 1) * SW], si_ps[:], AF.Exp,
                        scale=scl[:, s:s + 1], accum_out=s4[:, c:c + 1],
                    )
                ssum = work.tile([P, 1], F32, tag="ssum")
                nc.vector.reduce_sum(
                    out=ssum[:], in_=s4[:], axis=mybir.AxisListType.X
                )
                rs = work.tile([P, 1], F32, tag="rs")
                nc.vector.reciprocal(rs[:], ssum[:])

                # argmax over exp values (same ordering as probs)
                mx8 = work.tile([P, 8], F32, tag="mx8")
                nc.vector.max(out=mx8[:], in_=ex[:])
                mi8 = work.tile([P, 8], U32, tag="mi8")
                nc.vector.max_index(out=mi8[:], in_max=mx8[:], in_values=ex[:])
                bp = work.tile([P, E], F32, tag="bp")
                nc.gpsimd.indirect_dma_start(
                    out=bp[:], out_offset=None, in_=pr_d[:],
                    in_offset=bass.IndirectOffsetOnAxis(ap=mi8[:, :1], axis=0),
                )

                # probs = ex / sum  (in place, ACT Identity with AP scale)
                nc.scalar.activation(
                    ex[:], ex[:], AF.Identity, scale=rs[:, :1]
                )
                nc.sync.dma_start(probs_d[sr0:sr0 + P, :], ex[:])

                # late chain runs TWO subs late so the PE never stalls
                # on the gather's DMA semaphore
                pending.append((e0, s, sr0, bp))
                if len(pending) > 2:
                    emit_late(*pending.pop(0))

        for args in pending:
            emit_late(*args)

    nc.compile()
    return nc


def _prep_in_maps(inputs):
    full = {k: np.ascontiguousarray(np.asarray(v, dtype=np.float32))
            for k, v in inputs.items()}
    x = full.pop("inputs")
    protos = full["protos"]
    norms = np.maximum(
        np.linalg.norm(protos.astype(np.float64), axis=1, keepdims=True), 1e-8
    )
    full["pro_nT"] = np.ascontiguousarray(
        (protos.astype(np.float64) / norms).T.astype(np.float32)
    )
    xT = x.T  # [D_IN, B]
    in_maps = []
    for c in range(N_CORES):
        m = dict(full)
        m["inputsT"] = np.ascontiguousarray(xT[:, c * BC:(c + 1) * BC])
        in_maps.append(m)
    return in_maps


def kernel(**inputs):
    global _cached, last_results
    if _cached is None:
        _cached = _build()
    res = run_bass_kernel_spmd(_cached, _prep_in_maps(inputs),
                               list(range(N_CORES)))
    last_results = res
    probs = np.concatenate([r["probs"] for r in res.results], axis=0)
    abst = np.concatenate([r["abst"] for r in res.results], axis=0)
    hier = np.concatenate([r["hier"] for r in res.results], axis=0)
    return probs, abst, hier


def run_traced(inputs):
    """Profiled run (test-harness helper; requires the axon NTFF hook)."""
    global _cached
    if _cached is None:
        _cached = _build()
    return run_bass_kernel_spmd(_cached, _prep_in_maps(inputs),
                                list(range(N_CORES)), trace=True)
